# revision 1
# baseline (speedup 1.0000x reference)
"""Trainium2 Bass kernel for nn_LogicTreeConv2d.

Reference computation: unfold x (3x3, pad 1) -> per output-channel gather of 8
"leaf" patch rows -> depth-3 binary tree of relaxed logic gates, where each
node computes  c0 + c1*a + c2*b + c3*a*b  with coefficients
softmax(logits) @ GATE_COEF.

Strategy (8 NeuronCores, one SPMD program):
- Tensor-parallel over out_channels: core k owns oc [32k, 32k+32).  x is
  replicated; each core reads x once into SBUF and keeps it resident.
- SBUF x layout: partition p = hh*64 + b (hh = upper/lower 16-row half of H),
  per-partition frame [c][r][w] with r in [0,18) an 18-row halo window
  (global row hh*16 + r - 1, zero-padded out of range), w in [0,32)
  contiguous.  Every 3x3-shift leaf image is then a flat 512-element slice of
  the frame at offset c*576 + dy*32 + dx - 1(+guard), so tree math runs
  directly on views - no gather DMAs, no unfold materialization.
- W-direction pad: a shifted flat view bleeds one wrong element per row at
  w=0 (dx=0) or w=31 (dx=2).  Those two 16-element columns per level-0 node
  are recomputed with stride-32 column views (zero-substituted operands point
  at a zeroed strip), then overwrite the bled columns.
- Tree node = 2 fused custom DVE ops:
    u = (a*c3 + c2) * b        (AFFINE_MUL_REDUCE)
    o = (a*c1 + c0) + u        (AFFINE_THEN_ADD)
- Per-core leaf indices are runtime data: the per-leaf view offsets are an
  int32 input table, loaded into DVE registers (one reg_load per oc) and used
  as dynamic AP offsets, so the single compiled program serves all 8 cores.
- Gate-mixture coefficients are computed on device: exp on ScalarE, the
  16-gate contraction + softmax normalizer via one PE matmul against
  [ones | GATE_COEF], reciprocal + multiply on DVE, then a log-doubling
  SBUF->SBUF DMA broadcast to [128, 4*224] per-partition scalar columns.
"""

import numpy as np

import concourse.bacc as bacc
import concourse.mybir as mybir
from concourse import bass_utils
from concourse.bass import DynSlice
from concourse.tile import TileContext

# Problem constants (hardcoded per harness contract).
B, C, H, W = 64, 64, 32, 32
OC = 256
NCORES = 8
OCPC = OC // NCORES  # 32 out-channels per core
NL, NN = 8, 7  # leaves / nodes per tree

# SBUF frame layout.
GUARD = 1  # one zero word before the frame so dx-1 offsets stay >= 0
RW = 32  # row width
RPP = 18  # rows per frame (16 + 2 halo)
CSTR = RPP * RW  # 576 elements per channel
XDATA = C * CSTR  # 36864
TAILG = GUARD + XDATA  # tail guard word (c=63 last-row bleed target)
ZOFF = TAILG + 1  # zeroed strip for pad-substituted column views
XA = ZOFF + 16 * RW  # frame allocation: 37378 elements

GATE_COEF = np.array(
    [
        [0.0, 0.0, 0.0, 0.0],
        [0.0, 0.0, 0.0, 1.0],
        [0.0, 1.0, 0.0, -1.0],
        [0.0, 1.0, 0.0, 0.0],
        [0.0, 0.0, 1.0, -1.0],
        [0.0, 0.0, 1.0, 0.0],
        [0.0, 1.0, 1.0, -2.0],
        [0.0, 1.0, 1.0, -1.0],
        [1.0, -1.0, -1.0, 1.0],
        [1.0, -1.0, -1.0, 2.0],
        [1.0, 0.0, -1.0, 0.0],
        [1.0, 0.0, -1.0, 1.0],
        [1.0, -1.0, 0.0, 0.0],
        [1.0, -1.0, 0.0, 1.0],
        [1.0, 0.0, 0.0, -1.0],
        [1.0, 0.0, 0.0, 0.0],
    ],
    dtype=np.float32,
)

NK = OCPC * NN  # 224 (oc, node) coefficient columns per core

_cache: dict = {}


def _build_program():
    f32, i32 = mybir.dt.float32, mybir.dt.int32
    nc = bacc.Bacc(
        "TRN2",
        target_bir_lowering=False,
        debug=False,
        enable_asserts=False,
        num_devices=NCORES,
    )
    x_d = nc.dram_tensor("x", (B, C, H, W), f32, kind="ExternalInput").ap()
    lg_d = nc.dram_tensor("logits16", (16, NK), f32, kind="ExternalInput").ap()
    gc_d = nc.dram_tensor("gc5", (16, 5), f32, kind="ExternalInput").ap()
    off_d = nc.dram_tensor("offs", (1, OCPC * 24), i32, kind="ExternalInput").ap()
    y_d = nc.dram_tensor("y", (B, OCPC, H, W), f32, kind="ExternalOutput").ap()

    with TileContext(nc) as tc:
        with (
            tc.tile_pool(name="persist", bufs=1) as pp,
            tc.tile_pool(name="psum", bufs=1, space="PSUM") as psp,
        ):
            xov = pp.tile([128, XA], f32, tag="xov")
            coef = pp.tile([128, 4 * NK], f32, tag="coef")
            offs_t = pp.tile([1, OCPC * 24], i32, tag="offs")
            nc.sync.dma_start(out=offs_t[:], in_=off_d[:])

            # ---- coefficient pipeline: coef[p, j*NK + kk] = coef_j(oc,node)
            with tc.tile_pool(name="prep", bufs=1) as prp:
                lg_t = prp.tile([16, NK], f32, tag="lg")
                gc_t = prp.tile([16, 5], f32, tag="gc")
                nc.sync.dma_start(out=lg_t[:], in_=lg_d[:])
                nc.sync.dma_start(out=gc_t[:], in_=gc_d[:])
                e_t = prp.tile([16, NK], f32, tag="e")
                nc.scalar.activation(
                    e_t[:], lg_t[:], mybir.ActivationFunctionType.Exp
                )
                ps5 = psp.tile([5, NK], f32, tag="ps5")
                # rows: [sum(exp), ucoef0..3]
                nc.tensor.matmul(ps5[:], gc_t[:], e_t[:], start=True, stop=True)
                sb5 = prp.tile([5, NK], f32, tag="sb5")
                nc.scalar.copy(out=sb5[:], in_=ps5[:])
                rr = prp.tile([5, NK], f32, tag="rr")
                nc.vector.reciprocal(rr[0:1, :], sb5[0:1, :])
                nc.sync.dma_start(out=rr[1:2, :], in_=rr[0:1, :])
                nc.sync.dma_start(out=rr[2:4, :], in_=rr[0:2, :])
                nc.sync.dma_start(out=rr[4:5, :], in_=rr[0:1, :])
                c5 = prp.tile([5, NK], f32, tag="c5")
                # all 5 rows (partition starts must be aligned); row 0 = s/s
                nc.vector.tensor_mul(c5[0:5, :], sb5[0:5, :], rr[0:5, :])
                # gather 4 partition rows -> one 896-wide row, then log-double
                nc.sync.dma_start(
                    out=coef[0:1, :].rearrange("p (j k) -> p j k", j=4),
                    in_=c5[1:5, :],
                )
                n = 1
                while n < 128:
                    m = min(n, 128 - n)
                    nc.sync.dma_start(out=coef[n : n + m, :], in_=coef[0:m, :])
                    n += m

            # ---- x frame: pad memsets + halo'd loads
            nc.vector.memset(xov[:, 0:GUARD], 0.0)
            nc.vector.memset(xov[:, TAILG:XA], 0.0)
            body = xov[:, GUARD : GUARD + XDATA].rearrange(
                "p (c rw) -> p c rw", c=C
            )
            nc.vector.memset(body[0:64, :, 0:RW], 0.0)  # r=0 row, hh=0
            nc.vector.memset(body[64:128, :, 17 * RW : 18 * RW], 0.0)  # r=17, hh=1
            for c in range(C):
                for hh in (0, 1):
                    r0, h0 = (1, 0) if hh == 0 else (0, 15)
                    dst_off = GUARD + c * CSTR + r0 * RW
                    nc.sync.dma_start(
                        out=xov[hh * 64 : (hh + 1) * 64, dst_off : dst_off + 17 * RW],
                        in_=x_d[:, c, h0 : h0 + 17, :].rearrange("b h w -> b (h w)"),
                    )

            def cA(j, kk):
                return coef[:, j * NK + kk : j * NK + kk + 1]

            def col(sv):
                return xov[:, DynSlice(sv, 16, RW)]

            # ---- per-oc tree evaluation
            with (
                tc.tile_pool(name="work", bufs=2) as wp,
                tc.tile_pool(name="opool", bufs=4) as op,
                tc.tile_pool(name="ypool", bufs=3) as yp,
            ):
                for i in range(OCPC):
                    regs = [
                        nc.vector.alloc_register(f"off_{i}_{j}") for j in range(24)
                    ]
                    nc.vector.reg_load(regs, offs_t[0:1, i * 24 : (i + 1) * 24])
                    sv = [
                        nc.vector.snap(r, donate=True, min_val=0, max_val=ZOFF)
                        for r in regs
                    ]
                    lv = [xov[:, DynSlice(sv[j], 512)] for j in range(NL)]
                    kb = i * NN
                    os_ = []
                    pair = None
                    for n4 in range(4):
                        kk = kb + n4
                        scr = wp.tile([128, 1024], f32, tag="scr")
                        u = scr[:, 0:512]
                        fu = scr[:, 512:528]
                        fu2 = scr[:, 528:544]
                        jk = scr[:, 544:545]
                        a, b = lv[2 * n4], lv[2 * n4 + 1]
                        nc.vector.affine_mul_reduce(
                            out=u, accum_out=jk, in0=a, in1=b,
                            scale=cA(3, kk), bias=cA(2, kk),
                        )
                        if n4 % 2 == 0:
                            pair = op.tile([128, 1024], f32, tag="o")
                        base = (n4 % 2) * 512
                        on = pair[:, base : base + 512]
                        nc.vector.affine_then_add(
                            out=on, in0=a, in1=u, scale=cA(1, kk), bias=cA(0, kk)
                        )
                        # repair the two bled columns (w=0 / w=31)
                        a0, b0, a31, b31 = sv[8 + 4 * n4 : 12 + 4 * n4]
                        nc.vector.affine_mul_reduce(
                            out=fu, accum_out=jk, in0=col(a0), in1=col(b0),
                            scale=cA(3, kk), bias=cA(2, kk),
                        )
                        nc.vector.affine_then_add(
                            out=pair[:, DynSlice(base, 16, RW)],
                            in0=col(a0), in1=fu, scale=cA(1, kk), bias=cA(0, kk),
                        )
                        nc.vector.affine_mul_reduce(
                            out=fu2, accum_out=jk, in0=col(a31), in1=col(b31),
                            scale=cA(3, kk), bias=cA(2, kk),
                        )
                        nc.vector.affine_then_add(
                            out=pair[:, DynSlice(base + 31, 16, RW)],
                            in0=col(a31), in1=fu2, scale=cA(1, kk), bias=cA(0, kk),
                        )
                        os_.append(on)
                    ps_ = []
                    ppair = op.tile([128, 1024], f32, tag="o")
                    for m in range(2):
                        kk = kb + 4 + m
                        scr = wp.tile([128, 1024], f32, tag="scr")
                        u = scr[:, 0:512]
                        jk = scr[:, 544:545]
                        nc.vector.affine_mul_reduce(
                            out=u, accum_out=jk, in0=os_[2 * m], in1=os_[2 * m + 1],
                            scale=cA(3, kk), bias=cA(2, kk),
                        )
                        pm = ppair[:, m * 512 : (m + 1) * 512]
                        nc.vector.affine_then_add(
                            out=pm, in0=os_[2 * m], in1=u,
                            scale=cA(1, kk), bias=cA(0, kk),
                        )
                        ps_.append(pm)
                    kk = kb + 6
                    scr = wp.tile([128, 1024], f32, tag="scr")
                    u = scr[:, 0:512]
                    jk = scr[:, 544:545]
                    nc.vector.affine_mul_reduce(
                        out=u, accum_out=jk, in0=ps_[0], in1=ps_[1],
                        scale=cA(3, kk), bias=cA(2, kk),
                    )
                    yt = yp.tile([128, 512], f32, tag="y")
                    nc.vector.affine_then_add(
                        out=yt[:], in0=ps_[0], in1=u,
                        scale=cA(1, kk), bias=cA(0, kk),
                    )
                    for hh in (0, 1):
                        nc.sync.dma_start(
                            out=y_d[:, i, hh * 16 : (hh + 1) * 16, :],
                            in_=yt[hh * 64 : (hh + 1) * 64, :].rearrange(
                                "b (h w) -> b h w", h=16
                            ),
                        )
    nc.compile()
    return nc


def _host_inputs(x, logits, leaf_indices):
    """Per-core input maps. Host work is staging only: shard/transpose logits,
    translate leaf indices to frame offsets, append the ones column to the
    (constant) gate-coefficient table."""
    x = np.ascontiguousarray(np.asarray(x, dtype=np.float32))
    logits = np.asarray(logits, dtype=np.float32)
    li = np.asarray(leaf_indices).astype(np.int64)
    gc5 = np.concatenate(
        [np.ones((16, 1), np.float32), GATE_COEF], axis=1
    ).astype(np.float32)
    in_maps = []
    for k in range(NCORES):
        sh = logits[k * OCPC : (k + 1) * OCPC]  # (32, 7, 16)
        lg16 = np.ascontiguousarray(sh.reshape(NK, 16).T.astype(np.float32))
        lik = li[k * OCPC : (k + 1) * OCPC]  # (32, 8)
        offs = np.zeros((1, OCPC * 24), np.int32)
        for ocl in range(OCPC):
            base = ocl * 24
            ldx = []
            for j in range(NL):
                ki = int(lik[ocl, j])
                c, rem = divmod(ki, 9)
                dy, dx = divmod(rem, 3)
                o = c * CSTR + dy * RW + dx  # = GUARD + ... + (dx-1)
                assert 0 <= o and o + 512 <= ZOFF  # may touch tail guard word
                offs[0, base + j] = o
                ldx.append((o, dx))
            for n4 in range(4):
                oa, dxa = ldx[2 * n4]
                ob, dxb = ldx[2 * n4 + 1]
                offs[0, base + 8 + 4 * n4 + 0] = ZOFF if dxa == 0 else oa
                offs[0, base + 8 + 4 * n4 + 1] = ZOFF if dxb == 0 else ob
                offs[0, base + 8 + 4 * n4 + 2] = ZOFF if dxa == 2 else oa + 31
                offs[0, base + 8 + 4 * n4 + 3] = ZOFF if dxb == 2 else ob + 31
        in_maps.append({"x": x, "logits16": lg16, "gc5": gc5, "offs": offs})
    return in_maps


def kernel(x, logits, leaf_indices):
    if "nc" not in _cache:
        _cache["nc"] = _build_program()
    nc = _cache["nc"]
    in_maps = _host_inputs(x, logits, leaf_indices)
    res = bass_utils.run_bass_kernel_spmd(
        nc, in_maps, core_ids=list(range(NCORES))
    )
    y = np.concatenate(
        [res.results[k]["y"] for k in range(NCORES)], axis=1
    )
    _cache["last_results"] = res
    return y



# revision 3
# speedup vs baseline: 4.3431x; 4.3431x over previous
"""Trainium2 Bass kernel for nn_LogicTreeConv2d.

Reference computation: unfold x (3x3, pad 1) -> per output-channel gather of 8
"leaf" patch rows -> depth-3 binary tree of relaxed logic gates, where each
node computes  c0 + c1*a + c2*b + c3*a*b  with coefficients
softmax(logits) @ GATE_COEF.

The graded metric is wall-clock of kernel(), which over the axon tunnel is
dominated by host<->device transfer (measured ~65 MB/s up, ~48 MB/s down,
~100 ms fixed per dispatch).  Strategy:

- Data-parallel over batch: core k owns batches [8k, 8k+8).  x is uploaded
  exactly once (no replication): 17.8 MB instead of the 134 MB a
  tensor-parallel split would need.
- Output is produced on device as float16 (final rounding only; rel err
  ~5e-4 << 2e-2 tolerance), halving the download to 33.5 MB.  Host upcasts.
- The jitted SPMD executable is built once and cached; the donated output
  operand is kept device-resident (previous call's output buffer is fed
  back), so no per-call zero-buffer upload and no per-call retrace.
- Host pre-swizzles x into the exact SBUF frame layout (rows padded to
  width 34 with the conv zero columns, channel-major per 2-row slab) so the
  device ingests it with ONE fully contiguous DMA; the 2 halo rows per slab
  are built on-chip with partition-shifted SBUF->SBUF copies.
- Gate-mixture coefficients (softmax(logits)@GATE_COEF, 28 KB) are computed
  on host (staging-scale work) and broadcast to 128 partitions on device by
  log-doubling DMAs.

Device program per core (one SPMD program, 8 cores):
- SBUF frame [128, 8706] f32: partition p = b*16 + hs (b in [0,8), hs a
  2-row slab of H), per-partition frame [c][j][w] with j in [0,4) a 4-row
  halo window (global rows 2hs-1 .. 2hs+2) and w in [0,34) a zero-padded
  row.  Every 3x3-shift leaf image is a flat 68-element view at offset
  c*136 + dy*34 + dx; W-pad columns make the shifted views exact (junk
  columns stay finite and are stripped during the f32->f16 pack).
- Tree node = 2 fused custom DVE ops on [128, 68] views:
    u = (a*c3 + c2) * b        (AFFINE_MUL_REDUCE)
    o = (a*c1 + c0) + u        (AFFINE_THEN_ADD)
- Per-oc leaf offsets are runtime data (int32 table -> DVE registers ->
  dynamic AP offsets), so one compiled program serves any leaf_indices.
"""

import numpy as np

import concourse.bacc as bacc
import concourse.mybir as mybir
from concourse.bass import DynSlice
from concourse.tile import TileContext

# Problem constants (hardcoded per harness contract).
B, C, H, W = 64, 64, 32, 32
OC = 256
NCORES = 8
BPC = B // NCORES  # 8 batches per core
NL, NN = 8, 7  # leaves / nodes per tree
NK = OC * NN  # 1792 (oc, node) coefficient columns

# SBUF frame layout.
GUARD = 1  # one zero word before the frame so dy=dx=0 offsets stay >= 0
RW = 34  # padded row width (32 + 2 conv-zero columns)
RPP = 4  # rows per slab frame (2 + 2 halo)
CSTR = RPP * RW  # 136 elements per channel
XDATA = C * CSTR  # 8704
XA = GUARD + XDATA + 1  # +1 tail guard word
VLEN = 2 * RW  # 68-element leaf/node views
NPART = 128  # partitions: b*16 + hs

GATE_COEF = np.array(
    [
        [0.0, 0.0, 0.0, 0.0],
        [0.0, 0.0, 0.0, 1.0],
        [0.0, 1.0, 0.0, -1.0],
        [0.0, 1.0, 0.0, 0.0],
        [0.0, 0.0, 1.0, -1.0],
        [0.0, 0.0, 1.0, 0.0],
        [0.0, 1.0, 1.0, -2.0],
        [0.0, 1.0, 1.0, -1.0],
        [1.0, -1.0, -1.0, 1.0],
        [1.0, -1.0, -1.0, 2.0],
        [1.0, 0.0, -1.0, 0.0],
        [1.0, 0.0, -1.0, 1.0],
        [1.0, -1.0, 0.0, 0.0],
        [1.0, -1.0, 0.0, 1.0],
        [1.0, 0.0, 0.0, -1.0],
        [1.0, 0.0, 0.0, 0.0],
    ],
    dtype=np.float32,
)

_cache: dict = {}


def _build_program():
    f32, f16, i32 = mybir.dt.float32, mybir.dt.float16, mybir.dt.int32
    nc = bacc.Bacc(
        "TRN2",
        target_bir_lowering=False,
        debug=False,
        enable_asserts=False,
        num_devices=NCORES,
    )
    xm_d = nc.dram_tensor("xm", (NPART, C * VLEN), f32, kind="ExternalInput").ap()
    cf_d = nc.dram_tensor("coefs", (1, 4 * NK), f32, kind="ExternalInput").ap()
    off_d = nc.dram_tensor("offs", (1, OC * NL), i32, kind="ExternalInput").ap()
    y_d = nc.dram_tensor("y", (BPC, OC, H, W), f16, kind="ExternalOutput").ap()

    with TileContext(nc) as tc:
        with tc.tile_pool(name="persist", bufs=1) as pp:
            xov = pp.tile([NPART, XA], f32, tag="xov")
            coef = pp.tile([NPART, 4 * NK], f32, tag="coef")
            y16 = pp.tile([NPART, OC * 2 * W], f16, tag="y16")
            offs_t = pp.tile([1, OC * NL], i32, tag="offs")
            nc.sync.dma_start(out=offs_t[:], in_=off_d[:])

            # coefficient broadcast: [1, 7168] -> [128, 7168] by log-doubling
            nc.sync.dma_start(out=coef[0:1, :], in_=cf_d[:])
            n = 1
            while n < NPART:
                m = min(n, NPART - n)
                nc.sync.dma_start(out=coef[n : n + m, :], in_=coef[0:m, :])
                n += m

            # ---- x frame
            nc.vector.memset(xov[:], 0.0)
            body = xov[:, GUARD : GUARD + XDATA].rearrange(
                "p (c j wq) -> p c j wq", c=C, j=RPP
            )
            # main rows j=1,2: one contiguous DMA (host layout matches frame)
            nc.sync.dma_start(out=body[:, :, 1:3, :], in_=xm_d[:, :])
            # halo rows, per batch so copies never cross a batch boundary
            # (the boundary rows hs=0/j=0 and hs=15/j=3 stay memset-zero):
            for b in range(BPC):
                p0 = b * 16
                # j=0 (global row 2hs-1) = j=2 of partition p-1
                nc.sync.dma_start(
                    out=body[p0 + 1 : p0 + 16, :, 0:1, :],
                    in_=body[p0 : p0 + 15, :, 2:3, :],
                )
                # j=3 (global row 2hs+2) = j=1 of partition p+1
                nc.sync.dma_start(
                    out=body[p0 : p0 + 15, :, 3:4, :],
                    in_=body[p0 + 1 : p0 + 16, :, 1:2, :],
                )

            def cA(j, kk):
                return coef[:, j * NK + kk : j * NK + kk + 1]

            # ---- per-oc tree evaluation
            with (
                tc.tile_pool(name="work", bufs=2) as wp,
                tc.tile_pool(name="opool", bufs=3) as op,
                tc.tile_pool(name="ypool", bufs=3) as yp,
            ):
                for oc in range(OC):
                    regs = [
                        nc.vector.alloc_register(f"off_{oc}_{j}") for j in range(NL)
                    ]
                    nc.vector.reg_load(regs, offs_t[0:1, oc * NL : (oc + 1) * NL])
                    sv = [
                        nc.vector.snap(r, donate=True, min_val=0, max_val=XA - VLEN)
                        for r in regs
                    ]
                    lv = [xov[:, DynSlice(sv[j], VLEN)] for j in range(NL)]
                    kb = oc * NN
                    l0 = op.tile([NPART, 4 * VLEN], f32, tag="l0")
                    for n4 in range(4):
                        kk = kb + n4
                        scr = wp.tile([NPART, VLEN + 4], f32, tag="scr")
                        u = scr[:, 0:VLEN]
                        jk = scr[:, VLEN : VLEN + 1]
                        a, b = lv[2 * n4], lv[2 * n4 + 1]
                        nc.vector.affine_mul_reduce(
                            out=u, accum_out=jk, in0=a, in1=b,
                            scale=cA(3, kk), bias=cA(2, kk),
                        )
                        nc.vector.affine_then_add(
                            out=l0[:, n4 * VLEN : (n4 + 1) * VLEN],
                            in0=a, in1=u, scale=cA(1, kk), bias=cA(0, kk),
                        )
                    l1 = op.tile([NPART, 2 * VLEN], f32, tag="l1")
                    for m in range(2):
                        kk = kb + 4 + m
                        scr = wp.tile([NPART, VLEN + 4], f32, tag="scr")
                        u = scr[:, 0:VLEN]
                        jk = scr[:, VLEN : VLEN + 1]
                        a = l0[:, 2 * m * VLEN : (2 * m + 1) * VLEN]
                        b = l0[:, (2 * m + 1) * VLEN : (2 * m + 2) * VLEN]
                        nc.vector.affine_mul_reduce(
                            out=u, accum_out=jk, in0=a, in1=b,
                            scale=cA(3, kk), bias=cA(2, kk),
                        )
                        nc.vector.affine_then_add(
                            out=l1[:, m * VLEN : (m + 1) * VLEN],
                            in0=a, in1=u, scale=cA(1, kk), bias=cA(0, kk),
                        )
                    kk = kb + 6
                    scr = wp.tile([NPART, VLEN + 4], f32, tag="scr")
                    u = scr[:, 0:VLEN]
                    jk = scr[:, VLEN : VLEN + 1]
                    a, b = l1[:, 0:VLEN], l1[:, VLEN : 2 * VLEN]
                    nc.vector.affine_mul_reduce(
                        out=u, accum_out=jk, in0=a, in1=b,
                        scale=cA(3, kk), bias=cA(2, kk),
                    )
                    yt = yp.tile([NPART, VLEN], f32, tag="yt")
                    nc.vector.affine_then_add(
                        out=yt[:], in0=a, in1=u, scale=cA(1, kk), bias=cA(0, kk),
                    )
                    # f32 -> f16 convert + strip the 2 junk pad columns per row
                    ysl = y16[:, oc * 2 * W : (oc + 1) * 2 * W]
                    nc.scalar.copy(
                        out=ysl.rearrange("p (r w) -> p r w", r=2),
                        in_=yt.rearrange("p (r wq) -> p r wq", r=2)[:, :, 1 : 1 + W],
                    )
                    nc.sync.dma_start(
                        out=y_d[:, oc, :, :].rearrange("b h w -> b (h w)"),
                        in_=ysl,
                    )
    nc.compile()
    return nc


def _host_inputs(x, logits, leaf_indices):
    """Global (all-core) input arrays. Host work is staging only: swizzle x
    into the frame layout, mix the 16-gate coefficient table, translate leaf
    indices to frame offsets."""
    x = np.asarray(x, dtype=np.float32)
    logits = np.asarray(logits, dtype=np.float32)
    li = np.asarray(leaf_indices).astype(np.int64)

    # x -> (core*b*hs, c, j, w34) frame-main-rows layout, zero pad columns.
    if "xmbuf" not in _cache:
        _cache["xmbuf"] = np.zeros((B, H // 2, C, 2, RW), np.float32)
    xm = _cache["xmbuf"]
    # [b, hs, c, j, w'] = x[b, c, 2hs+j, w'-1]
    xm[..., 1 : 1 + W] = x.reshape(B, C, H // 2, 2, W).transpose(0, 2, 1, 3, 4)
    xm_g = xm.reshape(NCORES * NPART, C * VLEN)

    # softmax(logits) @ GATE_COEF in f64, laid out j-major: [j*NK + oc*NN + n]
    lg = logits.astype(np.float64)
    e = np.exp(lg - lg.max(axis=-1, keepdims=True))
    probs = e / e.sum(axis=-1, keepdims=True)
    cf = probs @ GATE_COEF.astype(np.float64)  # (OC, NN, 4)
    cf_flat = cf.transpose(2, 0, 1).reshape(1, 4 * NK).astype(np.float32)
    cf_g = np.ascontiguousarray(np.broadcast_to(cf_flat, (NCORES, 4 * NK)))

    # leaf index ki = c*9 + dy*3 + dx -> view offset c*136 + dy*34 + dx
    c_, rem = np.divmod(li, 9)
    dy, dx = np.divmod(rem, 3)
    off = (c_ * CSTR + dy * RW + dx).astype(np.int32).reshape(1, OC * NL)
    off_g = np.ascontiguousarray(np.broadcast_to(off, (NCORES, OC * NL)))
    return xm_g, cf_g, off_g


def _make_runner(nc):
    """Cached jitted SPMD executable over the 8 cores — same _bass_exec_p
    custom-call path as bass_utils.run_bass_kernel_spmd's axon redirect,
    minus the per-call retrace / input concat / zero-buffer upload."""
    import jax
    from jax.experimental.shard_map import shard_map
    from jax.sharding import Mesh, NamedSharding, PartitionSpec as P

    from concourse import bass2jax

    bass2jax.install_neuronx_cc_hook()

    partition_name = (
        nc.partition_id_tensor.name if nc.partition_id_tensor else None
    )
    in_names, out_names, out_avals = [], [], []
    for alloc in nc.m.functions[0].allocations:
        if not isinstance(alloc, mybir.MemoryLocationSet):
            continue
        name = alloc.memorylocations[0].name
        if alloc.kind == "ExternalInput":
            if name != partition_name:
                in_names.append(name)
        elif alloc.kind == "ExternalOutput":
            out_names.append(name)
            out_avals.append(
                jax.core.ShapedArray(
                    tuple(alloc.tensor_shape), mybir.dt.np(alloc.dtype)
                )
            )
    n_params = len(in_names)
    all_names = list(in_names) + list(out_names)
    if partition_name is not None:
        all_names.append(partition_name)

    def _body(*args):
        operands = list(args)
        if partition_name is not None:
            operands.append(bass2jax.partition_id_tensor())
        outs = bass2jax._bass_exec_p.bind(
            *operands,
            out_avals=tuple(out_avals),
            in_names=tuple(all_names),
            out_names=tuple(out_names),
            lowering_input_output_aliases=(),
            sim_require_finite=True,
            sim_require_nnan=True,
            nc=nc,
        )
        return tuple(outs)

    devices = jax.devices()[:NCORES]
    mesh = Mesh(np.asarray(devices), ("core",))
    n_ops = n_params + len(out_names)
    donate = tuple(range(n_params, n_ops))
    sharded = jax.jit(
        shard_map(
            _body,
            mesh=mesh,
            in_specs=(P("core"),) * n_ops,
            out_specs=(P("core"),) * len(out_names),
            check_rep=False,
        ),
        donate_argnums=donate,
        keep_unused=True,
    )
    ysh = NamedSharding(mesh, P("core"))
    return sharded, in_names, ysh


def kernel(x, logits, leaf_indices):
    import jax

    if "nc" not in _cache:
        _cache["nc"] = _build_program()
        _cache["runner"] = _make_runner(_cache["nc"])
    nc = _cache["nc"]
    sharded, in_names, ysh = _cache["runner"]

    xm_g, cf_g, off_g = _host_inputs(x, logits, leaf_indices)
    by_name = {"xm": xm_g, "coefs": cf_g, "offs": off_g}
    args = [by_name[n] for n in in_names]

    if "ydev" not in _cache:
        _cache["ydev"] = jax.device_put(
            np.zeros((B, OC, H, W), np.float16), ysh
        )
    outs = sharded(*args, _cache["ydev"])
    _cache["ydev"] = outs[0]
    y = np.asarray(outs[0]).astype(np.float32)
    return y


# revision 6
# speedup vs baseline: 4.4735x; 1.0300x over previous
"""Trainium2 Bass kernel for nn_LogicTreeConv2d.

Reference computation: unfold x (3x3, pad 1) -> per output-channel gather of 8
"leaf" patch rows -> depth-3 binary tree of relaxed logic gates, where each
node computes  c0 + c1*a + c2*b + c3*a*b  with coefficients
softmax(logits) @ GATE_COEF.

The graded metric is wall-clock of kernel(), which over the axon tunnel is
dominated by host<->device transfer (measured ~65 MB/s up, ~48 MB/s down,
~100 ms fixed per dispatch).  Strategy:

- Data-parallel over batch: core k owns batches [8k, 8k+8).  x is uploaded
  exactly once (no replication): 17.8 MB instead of the 134 MB a
  tensor-parallel split would need.
- Output is produced on device as float16 (final rounding only; rel err
  ~5e-4 << 2e-2 tolerance), halving the download to 33.5 MB.  Host upcasts.
- The jitted SPMD executable is built once and cached; the donated output
  operand is kept device-resident (previous call's output buffer is fed
  back), so no per-call zero-buffer upload and no per-call retrace.
- Host pre-swizzles x into the exact SBUF frame layout (rows padded to
  width 34 with the conv zero columns, channel-major per 2-row slab) so the
  device ingests it with ONE fully contiguous DMA; the 2 halo rows per slab
  are built on-chip with partition-shifted SBUF->SBUF copies.
- Gate-mixture coefficients (softmax(logits)@GATE_COEF, 28 KB) are computed
  on host (staging-scale work) and broadcast to 128 partitions on device by
  log-doubling DMAs.

Device program per core (one SPMD program, 8 cores):
- SBUF frame [128, 8706] f32: partition p = b*16 + hs (b in [0,8), hs a
  2-row slab of H), per-partition frame [c][j][w] with j in [0,4) a 4-row
  halo window (global rows 2hs-1 .. 2hs+2) and w in [0,34) a zero-padded
  row.  Every 3x3-shift leaf image is a flat 68-element view at offset
  c*136 + dy*34 + dx; W-pad columns make the shifted views exact (junk
  columns stay finite and are stripped during the f32->f16 pack).
- Tree node = 2 fused custom DVE ops on [128, 68] views:
    u = (a*c3 + c2) * b        (AFFINE_MUL_REDUCE)
    o = (a*c1 + c0) + u        (AFFINE_THEN_ADD)
- Per-oc leaf offsets are runtime data (int32 table -> DVE registers ->
  dynamic AP offsets), so one compiled program serves any leaf_indices.
"""

import numpy as np

import concourse.bacc as bacc
import concourse.mybir as mybir
from concourse.bass import DynSlice
from concourse.tile import TileContext

# Problem constants (hardcoded per harness contract).
B, C, H, W = 64, 64, 32, 32
OC = 256
NCORES = 8
BPC = B // NCORES  # 8 batches per core
NL, NN = 8, 7  # leaves / nodes per tree
NK = OC * NN  # 1792 (oc, node) coefficient columns

# SBUF frame layout.
GUARD = 1  # one zero word before the frame so dy=dx=0 offsets stay >= 0
RW = 34  # padded row width (32 + 2 conv-zero columns)
RPP = 4  # rows per slab frame (2 + 2 halo)
CSTR = RPP * RW  # 136 elements per channel
XDATA = C * CSTR  # 8704
XA = GUARD + XDATA + 1  # +1 tail guard word
VLEN = 2 * RW  # 68-element leaf/node views
NPART = 128  # partitions: b*16 + hs

GATE_COEF = np.array(
    [
        [0.0, 0.0, 0.0, 0.0],
        [0.0, 0.0, 0.0, 1.0],
        [0.0, 1.0, 0.0, -1.0],
        [0.0, 1.0, 0.0, 0.0],
        [0.0, 0.0, 1.0, -1.0],
        [0.0, 0.0, 1.0, 0.0],
        [0.0, 1.0, 1.0, -2.0],
        [0.0, 1.0, 1.0, -1.0],
        [1.0, -1.0, -1.0, 1.0],
        [1.0, -1.0, -1.0, 2.0],
        [1.0, 0.0, -1.0, 0.0],
        [1.0, 0.0, -1.0, 1.0],
        [1.0, -1.0, 0.0, 0.0],
        [1.0, -1.0, 0.0, 1.0],
        [1.0, 0.0, 0.0, -1.0],
        [1.0, 0.0, 0.0, 0.0],
    ],
    dtype=np.float32,
)

_cache: dict = {}


def _build_program():
    f32, f16, i32 = mybir.dt.float32, mybir.dt.float16, mybir.dt.int32
    nc = bacc.Bacc(
        "TRN2",
        target_bir_lowering=False,
        debug=False,
        enable_asserts=False,
        num_devices=NCORES,
    )
    xm_d = nc.dram_tensor("xm", (NPART, C * VLEN), f16, kind="ExternalInput").ap()
    cf_d = nc.dram_tensor("coefs", (1, 4 * NK), f32, kind="ExternalInput").ap()
    off_d = nc.dram_tensor("offs", (1, OC * NL), i32, kind="ExternalInput").ap()
    y_d = nc.dram_tensor("y", (BPC, OC, H, W), f16, kind="ExternalOutput").ap()

    with TileContext(nc) as tc:
        with tc.tile_pool(name="persist", bufs=1) as pp:
            xov = pp.tile([NPART, XA], f32, tag="xov")
            coef = pp.tile([NPART, 4 * NK], f32, tag="coef")
            y16 = pp.tile([NPART, OC * 2 * W], f16, tag="y16")
            offs_t = pp.tile([1, OC * NL], i32, tag="offs")
            nc.sync.dma_start(out=offs_t[:], in_=off_d[:])

            # coefficient broadcast: [1, 7168] -> [128, 7168] by log-doubling
            nc.sync.dma_start(out=coef[0:1, :], in_=cf_d[:])
            n = 1
            while n < NPART:
                m = min(n, NPART - n)
                nc.sync.dma_start(out=coef[n : n + m, :], in_=coef[0:m, :])
                n += m

            # ---- x frame
            xh = pp.tile([NPART, C * VLEN], f16, tag="xh")
            nc.sync.dma_start(out=xh[:], in_=xm_d[:, :])
            nc.vector.memset(xov[:], 0.0)
            body = xov[:, GUARD : GUARD + XDATA].rearrange(
                "p (c j wq) -> p c j wq", c=C, j=RPP
            )
            # main rows j=1,2: f16 -> f32 convert into the frame layout
            nc.scalar.copy(
                out=body[:, :, 1:3, :].rearrange("p c j wq -> p c (j wq)"),
                in_=xh.rearrange("p (c v) -> p c v", c=C),
            )
            # halo rows, per batch so copies never cross a batch boundary
            # (the boundary rows hs=0/j=0 and hs=15/j=3 stay memset-zero):
            for b in range(BPC):
                p0 = b * 16
                # j=0 (global row 2hs-1) = j=2 of partition p-1
                nc.sync.dma_start(
                    out=body[p0 + 1 : p0 + 16, :, 0:1, :],
                    in_=body[p0 : p0 + 15, :, 2:3, :],
                )
                # j=3 (global row 2hs+2) = j=1 of partition p+1
                nc.sync.dma_start(
                    out=body[p0 : p0 + 15, :, 3:4, :],
                    in_=body[p0 + 1 : p0 + 16, :, 1:2, :],
                )

            def cA(j, kk):
                return coef[:, j * NK + kk : j * NK + kk + 1]

            # ---- per-oc tree evaluation
            with (
                tc.tile_pool(name="work", bufs=2) as wp,
                tc.tile_pool(name="opool", bufs=3) as op,
                tc.tile_pool(name="ypool", bufs=3) as yp,
            ):
                for oc in range(OC):
                    regs = [
                        nc.vector.alloc_register(f"off_{oc}_{j}") for j in range(NL)
                    ]
                    nc.vector.reg_load(regs, offs_t[0:1, oc * NL : (oc + 1) * NL])
                    sv = [
                        nc.vector.snap(r, donate=True, min_val=0, max_val=XA - VLEN)
                        for r in regs
                    ]
                    lv = [xov[:, DynSlice(sv[j], VLEN)] for j in range(NL)]
                    kb = oc * NN
                    l0 = op.tile([NPART, 4 * VLEN], f32, tag="l0")
                    for n4 in range(4):
                        kk = kb + n4
                        scr = wp.tile([NPART, VLEN + 4], f32, tag="scr")
                        u = scr[:, 0:VLEN]
                        jk = scr[:, VLEN : VLEN + 1]
                        a, b = lv[2 * n4], lv[2 * n4 + 1]
                        nc.vector.affine_mul_reduce(
                            out=u, accum_out=jk, in0=a, in1=b,
                            scale=cA(3, kk), bias=cA(2, kk),
                        )
                        nc.vector.affine_then_add(
                            out=l0[:, n4 * VLEN : (n4 + 1) * VLEN],
                            in0=a, in1=u, scale=cA(1, kk), bias=cA(0, kk),
                        )
                    l1 = op.tile([NPART, 2 * VLEN], f32, tag="l1")
                    for m in range(2):
                        kk = kb + 4 + m
                        scr = wp.tile([NPART, VLEN + 4], f32, tag="scr")
                        u = scr[:, 0:VLEN]
                        jk = scr[:, VLEN : VLEN + 1]
                        a = l0[:, 2 * m * VLEN : (2 * m + 1) * VLEN]
                        b = l0[:, (2 * m + 1) * VLEN : (2 * m + 2) * VLEN]
                        nc.vector.affine_mul_reduce(
                            out=u, accum_out=jk, in0=a, in1=b,
                            scale=cA(3, kk), bias=cA(2, kk),
                        )
                        nc.vector.affine_then_add(
                            out=l1[:, m * VLEN : (m + 1) * VLEN],
                            in0=a, in1=u, scale=cA(1, kk), bias=cA(0, kk),
                        )
                    kk = kb + 6
                    scr = wp.tile([NPART, VLEN + 4], f32, tag="scr")
                    u = scr[:, 0:VLEN]
                    jk = scr[:, VLEN : VLEN + 1]
                    a, b = l1[:, 0:VLEN], l1[:, VLEN : 2 * VLEN]
                    nc.vector.affine_mul_reduce(
                        out=u, accum_out=jk, in0=a, in1=b,
                        scale=cA(3, kk), bias=cA(2, kk),
                    )
                    yt = yp.tile([NPART, VLEN], f32, tag="yt")
                    nc.vector.affine_then_add(
                        out=yt[:], in0=a, in1=u, scale=cA(1, kk), bias=cA(0, kk),
                    )
                    # f32 -> f16 convert + strip the 2 junk pad columns per row
                    ysl = y16[:, oc * 2 * W : (oc + 1) * 2 * W]
                    nc.scalar.copy(
                        out=ysl.rearrange("p (r w) -> p r w", r=2),
                        in_=yt.rearrange("p (r wq) -> p r wq", r=2)[:, :, 1 : 1 + W],
                    )
                    nc.sync.dma_start(
                        out=y_d[:, oc, :, :].rearrange("b h w -> b (h w)"),
                        in_=ysl,
                    )
    nc.compile()
    return nc


def _host_inputs(x, logits, leaf_indices):
    """Global (all-core) input arrays. Host work is staging only: swizzle x
    into the frame layout, mix the 16-gate coefficient table, translate leaf
    indices to frame offsets."""
    x = np.asarray(x, dtype=np.float32)
    logits = np.asarray(logits, dtype=np.float32)
    li = np.asarray(leaf_indices).astype(np.int64)

    # x -> (core*b*hs, c, j, w34) frame-main-rows layout, zero pad columns.
    if "xmbuf" not in _cache:
        _cache["xmbuf"] = np.zeros((B, H // 2, C, 2, RW), np.float16)
    xm = _cache["xmbuf"]
    # [b, hs, c, j, w'] = x[b, c, 2hs+j, w'-1]
    xm[..., 1 : 1 + W] = x.reshape(B, C, H // 2, 2, W).transpose(0, 2, 1, 3, 4)
    xm_g = xm.reshape(NCORES * NPART, C * VLEN)

    # softmax(logits) @ GATE_COEF in f64, laid out j-major: [j*NK + oc*NN + n]
    lg = logits.astype(np.float64)
    e = np.exp(lg - lg.max(axis=-1, keepdims=True))
    probs = e / e.sum(axis=-1, keepdims=True)
    cf = probs @ GATE_COEF.astype(np.float64)  # (OC, NN, 4)
    cf_flat = cf.transpose(2, 0, 1).reshape(1, 4 * NK).astype(np.float32)
    cf_g = np.ascontiguousarray(np.broadcast_to(cf_flat, (NCORES, 4 * NK)))

    # leaf index ki = c*9 + dy*3 + dx -> view offset c*136 + dy*34 + dx
    c_, rem = np.divmod(li, 9)
    dy, dx = np.divmod(rem, 3)
    off = (c_ * CSTR + dy * RW + dx).astype(np.int32).reshape(1, OC * NL)
    off_g = np.ascontiguousarray(np.broadcast_to(off, (NCORES, OC * NL)))
    return xm_g, cf_g, off_g


def _make_runner(nc):
    """Cached jitted SPMD executable over the 8 cores — same _bass_exec_p
    custom-call path as bass_utils.run_bass_kernel_spmd's axon redirect,
    minus the per-call retrace / input concat / zero-buffer upload."""
    import jax
    from jax.experimental.shard_map import shard_map
    from jax.sharding import Mesh, NamedSharding, PartitionSpec as P

    from concourse import bass2jax

    bass2jax.install_neuronx_cc_hook()

    partition_name = (
        nc.partition_id_tensor.name if nc.partition_id_tensor else None
    )
    in_names, out_names, out_avals = [], [], []
    for alloc in nc.m.functions[0].allocations:
        if not isinstance(alloc, mybir.MemoryLocationSet):
            continue
        name = alloc.memorylocations[0].name
        if alloc.kind == "ExternalInput":
            if name != partition_name:
                in_names.append(name)
        elif alloc.kind == "ExternalOutput":
            out_names.append(name)
            out_avals.append(
                jax.core.ShapedArray(
                    tuple(alloc.tensor_shape), mybir.dt.np(alloc.dtype)
                )
            )
    n_params = len(in_names)
    all_names = list(in_names) + list(out_names)
    if partition_name is not None:
        all_names.append(partition_name)

    def _body(*args):
        operands = list(args)
        if partition_name is not None:
            operands.append(bass2jax.partition_id_tensor())
        outs = bass2jax._bass_exec_p.bind(
            *operands,
            out_avals=tuple(out_avals),
            in_names=tuple(all_names),
            out_names=tuple(out_names),
            lowering_input_output_aliases=(),
            sim_require_finite=True,
            sim_require_nnan=True,
            nc=nc,
        )
        return tuple(outs)

    devices = jax.devices()[:NCORES]
    mesh = Mesh(np.asarray(devices), ("core",))
    n_ops = n_params + len(out_names)
    donate = tuple(range(n_params, n_ops))
    sharded = jax.jit(
        shard_map(
            _body,
            mesh=mesh,
            in_specs=(P("core"),) * n_ops,
            out_specs=(P("core"),) * len(out_names),
            check_rep=False,
        ),
        donate_argnums=donate,
        keep_unused=True,
    )
    ysh = NamedSharding(mesh, P("core"))
    return sharded, in_names, ysh


def kernel(x, logits, leaf_indices):
    import jax

    if "nc" not in _cache:
        _cache["nc"] = _build_program()
        _cache["runner"] = _make_runner(_cache["nc"])
    nc = _cache["nc"]
    sharded, in_names, ysh = _cache["runner"]

    xm_g, cf_g, off_g = _host_inputs(x, logits, leaf_indices)
    by_name = {"xm": xm_g, "coefs": cf_g, "offs": off_g}
    args = [by_name[n] for n in in_names]

    if "ydev" not in _cache:
        _cache["ydev"] = jax.device_put(
            np.zeros((B, OC, H, W), np.float16), ysh
        )
    outs = sharded(*args, _cache["ydev"])
    _cache["ydev"] = outs[0]
    y = np.asarray(outs[0]).astype(np.float32)
    return y


# revision 7
# speedup vs baseline: 5.0716x; 1.1337x over previous
"""Trainium2 Bass kernel for nn_LogicTreeConv2d.

Reference computation: unfold x (3x3, pad 1) -> per output-channel gather of 8
"leaf" patch rows -> depth-3 binary tree of relaxed logic gates, where each
node computes  c0 + c1*a + c2*b + c3*a*b  with coefficients
softmax(logits) @ GATE_COEF.

The graded metric is wall-clock of kernel(), which over the axon tunnel is
dominated by host<->device transfer (measured ~65 MB/s up, ~48 MB/s down,
~100 ms fixed per dispatch).  Strategy:

- Data-parallel over batch: core k owns batches [8k, 8k+8).  x is uploaded
  exactly once (no replication): 17.8 MB instead of the 134 MB a
  tensor-parallel split would need.
- Output is produced on device as float16 (final rounding only; rel err
  ~5e-4 << 2e-2 tolerance), halving the download to 33.5 MB.  Host upcasts.
- The jitted SPMD executable is built once and cached; the donated output
  operand is kept device-resident (previous call's output buffer is fed
  back), so no per-call zero-buffer upload and no per-call retrace.
- Host pre-swizzles x into the exact SBUF frame layout (rows padded to
  width 34 with the conv zero columns, channel-major per 2-row slab) so the
  device ingests it with ONE fully contiguous DMA; the 2 halo rows per slab
  are built on-chip with partition-shifted SBUF->SBUF copies.
- Gate-mixture coefficients (softmax(logits)@GATE_COEF, 28 KB) are computed
  on host (staging-scale work) and broadcast to 128 partitions on device by
  log-doubling DMAs.

Device program per core (one SPMD program, 8 cores):
- SBUF frame [128, 8706] f32: partition p = b*16 + hs (b in [0,8), hs a
  2-row slab of H), per-partition frame [c][j][w] with j in [0,4) a 4-row
  halo window (global rows 2hs-1 .. 2hs+2) and w in [0,34) a zero-padded
  row.  Every 3x3-shift leaf image is a flat 68-element view at offset
  c*136 + dy*34 + dx; W-pad columns make the shifted views exact (junk
  columns stay finite and are stripped during the f32->f16 pack).
- Tree node = 2 fused custom DVE ops on [128, 68] views:
    u = (a*c3 + c2) * b        (AFFINE_MUL_REDUCE)
    o = (a*c1 + c0) + u        (AFFINE_THEN_ADD)
- Per-oc leaf offsets are runtime data (int32 table -> DVE registers ->
  dynamic AP offsets), so one compiled program serves any leaf_indices.
"""

import numpy as np

import concourse.bacc as bacc
import concourse.mybir as mybir
from concourse.bass import DynSlice
from concourse.tile import TileContext

# Problem constants (hardcoded per harness contract).
B, C, H, W = 64, 64, 32, 32
OC = 256
NCORES = 8
BPC = B // NCORES  # 8 batches per core
NL, NN = 8, 7  # leaves / nodes per tree
NK = OC * NN  # 1792 (oc, node) coefficient columns

# SBUF frame layout.
GUARD = 1  # one zero word before the frame so dy=dx=0 offsets stay >= 0
RW = 34  # padded row width (32 + 2 conv-zero columns)
RPP = 4  # rows per slab frame (2 + 2 halo)
CSTR = RPP * RW  # 136 elements per channel
XDATA = C * CSTR  # 8704
XA = GUARD + XDATA + 1  # +1 tail guard word
VLEN = 2 * RW  # 68-element leaf/node views
NPART = 128  # partitions: b*16 + hs

GATE_COEF = np.array(
    [
        [0.0, 0.0, 0.0, 0.0],
        [0.0, 0.0, 0.0, 1.0],
        [0.0, 1.0, 0.0, -1.0],
        [0.0, 1.0, 0.0, 0.0],
        [0.0, 0.0, 1.0, -1.0],
        [0.0, 0.0, 1.0, 0.0],
        [0.0, 1.0, 1.0, -2.0],
        [0.0, 1.0, 1.0, -1.0],
        [1.0, -1.0, -1.0, 1.0],
        [1.0, -1.0, -1.0, 2.0],
        [1.0, 0.0, -1.0, 0.0],
        [1.0, 0.0, -1.0, 1.0],
        [1.0, -1.0, 0.0, 0.0],
        [1.0, -1.0, 0.0, 1.0],
        [1.0, 0.0, 0.0, -1.0],
        [1.0, 0.0, 0.0, 0.0],
    ],
    dtype=np.float32,
)

_cache: dict = {}


def _build_program():
    f32, f16, i32 = mybir.dt.float32, mybir.dt.float16, mybir.dt.int32
    nc = bacc.Bacc(
        "TRN2",
        target_bir_lowering=False,
        debug=False,
        enable_asserts=False,
        num_devices=NCORES,
    )
    xm_d = nc.dram_tensor("xm", (NPART, C * VLEN), f16, kind="ExternalInput").ap()
    cf_d = nc.dram_tensor("coefs", (1, 4 * NK), f32, kind="ExternalInput").ap()
    off_d = nc.dram_tensor("offs", (1, OC * NL), i32, kind="ExternalInput").ap()
    y_d = nc.dram_tensor("y", (BPC, OC, H, W), f16, kind="ExternalOutput").ap()

    with TileContext(nc) as tc:
        with tc.tile_pool(name="persist", bufs=1) as pp:
            xov = pp.tile([NPART, XA], f32, tag="xov")
            coef = pp.tile([NPART, 4 * NK], f32, tag="coef")
            y16 = pp.tile([NPART, OC * 2 * W], f16, tag="y16")
            offs_t = pp.tile([1, OC * NL], i32, tag="offs")
            nc.sync.dma_start(out=offs_t[:], in_=off_d[:])

            # coefficient broadcast: [1, 7168] -> [128, 7168] by log-doubling
            nc.sync.dma_start(out=coef[0:1, :], in_=cf_d[:])
            n = 1
            while n < NPART:
                m = min(n, NPART - n)
                nc.sync.dma_start(out=coef[n : n + m, :], in_=coef[0:m, :])
                n += m

            # ---- x frame
            xh = pp.tile([NPART, C * VLEN], f16, tag="xh")
            nc.sync.dma_start(out=xh[:], in_=xm_d[:, :])
            nc.vector.memset(xov[:], 0.0)
            body = xov[:, GUARD : GUARD + XDATA].rearrange(
                "p (c j wq) -> p c j wq", c=C, j=RPP
            )
            # main rows j=1,2: f16 -> f32 convert into the frame layout
            nc.scalar.copy(
                out=body[:, :, 1:3, :].rearrange("p c j wq -> p c (j wq)"),
                in_=xh.rearrange("p (c v) -> p c v", c=C),
            )
            # halo rows, per batch so copies never cross a batch boundary
            # (the boundary rows hs=0/j=0 and hs=15/j=3 stay memset-zero):
            for b in range(BPC):
                p0 = b * 16
                # j=0 (global row 2hs-1) = j=2 of partition p-1
                nc.sync.dma_start(
                    out=body[p0 + 1 : p0 + 16, :, 0:1, :],
                    in_=body[p0 : p0 + 15, :, 2:3, :],
                )
                # j=3 (global row 2hs+2) = j=1 of partition p+1
                nc.sync.dma_start(
                    out=body[p0 : p0 + 15, :, 3:4, :],
                    in_=body[p0 + 1 : p0 + 16, :, 1:2, :],
                )

            def cA(j, kk):
                return coef[:, j * NK + kk : j * NK + kk + 1]

            # ---- per-oc tree evaluation
            with (
                tc.tile_pool(name="work", bufs=2) as wp,
                tc.tile_pool(name="opool", bufs=3) as op,
                tc.tile_pool(name="ypool", bufs=3) as yp,
            ):
                for oc in range(OC):
                    regs = [
                        nc.vector.alloc_register(f"off_{oc}_{j}") for j in range(NL)
                    ]
                    nc.vector.reg_load(regs, offs_t[0:1, oc * NL : (oc + 1) * NL])
                    sv = [
                        nc.vector.snap(r, donate=True, min_val=0, max_val=XA - VLEN)
                        for r in regs
                    ]
                    lv = [xov[:, DynSlice(sv[j], VLEN)] for j in range(NL)]
                    kb = oc * NN
                    l0 = op.tile([NPART, 4 * VLEN], f32, tag="l0")
                    for n4 in range(4):
                        kk = kb + n4
                        scr = wp.tile([NPART, VLEN + 4], f32, tag="scr")
                        u = scr[:, 0:VLEN]
                        jk = scr[:, VLEN : VLEN + 1]
                        a, b = lv[2 * n4], lv[2 * n4 + 1]
                        nc.vector.affine_mul_reduce(
                            out=u, accum_out=jk, in0=a, in1=b,
                            scale=cA(3, kk), bias=cA(2, kk),
                        )
                        nc.vector.affine_then_add(
                            out=l0[:, n4 * VLEN : (n4 + 1) * VLEN],
                            in0=a, in1=u, scale=cA(1, kk), bias=cA(0, kk),
                        )
                    l1 = op.tile([NPART, 2 * VLEN], f32, tag="l1")
                    for m in range(2):
                        kk = kb + 4 + m
                        scr = wp.tile([NPART, VLEN + 4], f32, tag="scr")
                        u = scr[:, 0:VLEN]
                        jk = scr[:, VLEN : VLEN + 1]
                        a = l0[:, 2 * m * VLEN : (2 * m + 1) * VLEN]
                        b = l0[:, (2 * m + 1) * VLEN : (2 * m + 2) * VLEN]
                        nc.vector.affine_mul_reduce(
                            out=u, accum_out=jk, in0=a, in1=b,
                            scale=cA(3, kk), bias=cA(2, kk),
                        )
                        nc.vector.affine_then_add(
                            out=l1[:, m * VLEN : (m + 1) * VLEN],
                            in0=a, in1=u, scale=cA(1, kk), bias=cA(0, kk),
                        )
                    kk = kb + 6
                    scr = wp.tile([NPART, VLEN + 4], f32, tag="scr")
                    u = scr[:, 0:VLEN]
                    jk = scr[:, VLEN : VLEN + 1]
                    a, b = l1[:, 0:VLEN], l1[:, VLEN : 2 * VLEN]
                    nc.vector.affine_mul_reduce(
                        out=u, accum_out=jk, in0=a, in1=b,
                        scale=cA(3, kk), bias=cA(2, kk),
                    )
                    yt = yp.tile([NPART, VLEN], f32, tag="yt")
                    nc.vector.affine_then_add(
                        out=yt[:], in0=a, in1=u, scale=cA(1, kk), bias=cA(0, kk),
                    )
                    # f32 -> f16 convert + strip the 2 junk pad columns per row
                    ysl = y16[:, oc * 2 * W : (oc + 1) * 2 * W]
                    nc.scalar.copy(
                        out=ysl.rearrange("p (r w) -> p r w", r=2),
                        in_=yt.rearrange("p (r wq) -> p r wq", r=2)[:, :, 1 : 1 + W],
                    )
                    nc.sync.dma_start(
                        out=y_d[:, oc, :, :].rearrange("b h w -> b (h w)"),
                        in_=ysl,
                    )
    nc.compile()
    return nc


def _host_inputs(x, logits, leaf_indices):
    """Global (all-core) input arrays. Host work is staging only: swizzle x
    into the frame layout, mix the 16-gate coefficient table, translate leaf
    indices to frame offsets."""
    x = np.asarray(x, dtype=np.float32)
    logits = np.asarray(logits, dtype=np.float32)
    li = np.asarray(leaf_indices).astype(np.int64)

    # x -> (core*b*hs, c, j, w34) frame-main-rows layout, zero pad columns.
    if "xmbuf" not in _cache:
        _cache["xmbuf"] = np.zeros((B, H // 2, C, 2, RW), np.float16)
    xm = _cache["xmbuf"]
    # [b, hs, c, j, w'] = x[b, c, 2hs+j, w'-1]
    xm[..., 1 : 1 + W] = x.reshape(B, C, H // 2, 2, W).transpose(0, 2, 1, 3, 4)
    xm_g = xm.reshape(NCORES * NPART, C * VLEN)

    # softmax(logits) @ GATE_COEF in f64, laid out j-major: [j*NK + oc*NN + n]
    lg = logits.astype(np.float64)
    e = np.exp(lg - lg.max(axis=-1, keepdims=True))
    probs = e / e.sum(axis=-1, keepdims=True)
    cf = probs @ GATE_COEF.astype(np.float64)  # (OC, NN, 4)
    cf_flat = cf.transpose(2, 0, 1).reshape(1, 4 * NK).astype(np.float32)
    cf_g = np.ascontiguousarray(np.broadcast_to(cf_flat, (NCORES, 4 * NK)))

    # leaf index ki = c*9 + dy*3 + dx -> view offset c*136 + dy*34 + dx
    c_, rem = np.divmod(li, 9)
    dy, dx = np.divmod(rem, 3)
    off = (c_ * CSTR + dy * RW + dx).astype(np.int32).reshape(1, OC * NL)
    off_g = np.ascontiguousarray(np.broadcast_to(off, (NCORES, OC * NL)))
    return xm_g, cf_g, off_g


def _make_runner(nc):
    """Cached jitted SPMD executable over the 8 cores — same _bass_exec_p
    custom-call path as bass_utils.run_bass_kernel_spmd's axon redirect,
    minus the per-call retrace / input concat / zero-buffer upload."""
    import jax
    from jax.experimental.shard_map import shard_map
    from jax.sharding import Mesh, NamedSharding, PartitionSpec as P

    from concourse import bass2jax

    bass2jax.install_neuronx_cc_hook()

    partition_name = (
        nc.partition_id_tensor.name if nc.partition_id_tensor else None
    )
    in_names, out_names, out_avals = [], [], []
    for alloc in nc.m.functions[0].allocations:
        if not isinstance(alloc, mybir.MemoryLocationSet):
            continue
        name = alloc.memorylocations[0].name
        if alloc.kind == "ExternalInput":
            if name != partition_name:
                in_names.append(name)
        elif alloc.kind == "ExternalOutput":
            out_names.append(name)
            out_avals.append(
                jax.core.ShapedArray(
                    tuple(alloc.tensor_shape), mybir.dt.np(alloc.dtype)
                )
            )
    n_params = len(in_names)
    all_names = list(in_names) + list(out_names)
    if partition_name is not None:
        all_names.append(partition_name)

    def _body(*args):
        operands = list(args)
        if partition_name is not None:
            operands.append(bass2jax.partition_id_tensor())
        outs = bass2jax._bass_exec_p.bind(
            *operands,
            out_avals=tuple(out_avals),
            in_names=tuple(all_names),
            out_names=tuple(out_names),
            lowering_input_output_aliases=(),
            sim_require_finite=True,
            sim_require_nnan=True,
            nc=nc,
        )
        return tuple(outs)

    devices = jax.devices()[:NCORES]
    mesh = Mesh(np.asarray(devices), ("core",))
    n_ops = n_params + len(out_names)
    donate = tuple(range(n_params, n_ops))
    sharded = jax.jit(
        shard_map(
            _body,
            mesh=mesh,
            in_specs=(P("core"),) * n_ops,
            out_specs=(P("core"),) * len(out_names),
            check_rep=False,
        ),
        donate_argnums=donate,
        keep_unused=True,
    )
    ysh = NamedSharding(mesh, P("core"))
    return sharded, in_names, ysh


def kernel(x, logits, leaf_indices):
    import jax
    from concurrent.futures import ThreadPoolExecutor

    if "nc" not in _cache:
        _cache["nc"] = _build_program()
        _cache["runner"] = _make_runner(_cache["nc"])
        _cache["pool"] = ThreadPoolExecutor(max_workers=NCORES)
    sharded, in_names, ysh = _cache["runner"]

    xm_g, cf_g, off_g = _host_inputs(x, logits, leaf_indices)
    by_name = {"xm": xm_g, "coefs": cf_g, "offs": off_g}
    args = [by_name[n] for n in in_names]

    if "ydev" not in _cache:
        _cache["ydev"] = jax.device_put(
            np.zeros((B, OC, H, W), np.float16), ysh
        )
    outs = sharded(*args, _cache["ydev"])
    _cache["ydev"] = outs[0]

    # Fetch the 8 per-core shards concurrently (multi-stream beats the
    # single-stream tunnel rate) and upcast f16 -> f32 as each arrives.
    y = np.empty((B, OC, H, W), np.float32)

    def _fetch(shard):
        b0 = shard.index[0].start or 0
        y[b0 : b0 + BPC] = np.asarray(shard.data)

    list(_cache["pool"].map(_fetch, outs[0].addressable_shards))
    return y
